# revision 10
# baseline (speedup 1.0000x reference)
"""Trainium2 Bass kernel for batched multi-head self-attention.

Problem: x[8,1024,768], w_qkv[768,2304], b_qkv[2304] ->
         out[8,1024,768]  (12 heads, head_dim 64, scale 768**-0.5)

Sharding: data-parallel over batch; each of the 8 NeuronCores processes one
batch element end-to-end (no collectives).

Per-core pipeline, software-pipelined so the PE never waits on the Scalar
engine's exp (which otherwise rate-limits attention):
  1. Host pre-work: transpose x[b] -> xT16 [768,1024] fp16; permute w_qkv
     columns so QK features are grouped per head-pair and V features
     head-major with a ones column per head (softmax denominators fall out
     of the PV matmul).
  2. QK projection in [feature, token] orientation (fp16) -> Q^T/K^T tiles;
     V projection in [token, feature] orientation (fp16) -> [V|1] tiles.
  3. Attention runs as 12 chunks c=(pair, q-half).  Steady state issues, per
     chunk period: energy matmuls + exp for chunk c interleaved (per k-tile)
     with the PV matmuls of chunk c-1, so exp(c-1) results are ready exactly
     when PV(c-1) consumes them and the Tensor engine stays saturated (and
     the HAM clock gate stays at 2.4 GHz).  exp is written as fp16, making
     the PV moving operand full-rate.  The PV output [d+1, q] (denominator
     row included) is copied to fp16, PE-transposed back to [q, d] (fp16,
     1 cycle/row), normalized with one batched reciprocal per head, and the
     finished 128-token x 2-head block is DMAed out per chunk.
"""

import numpy as np

import concourse.mybir as mybir
import concourse.tile as tile
from concourse import bacc
from concourse.bass_utils import run_bass_kernel_spmd
from concourse.masks import make_identity

B, NT, D, H, HD = 8, 1024, 768, 12, 64
KC = D // 128          # 6 contraction chunks
NPAIR = H // 2         # 6 head pairs
NCH = 2 * NPAIR        # 12 chunks: (pair, q-half)
SCALE = float(D) ** -0.5
F32 = mybir.dt.float32
FP16 = mybir.dt.float16
VP_W = H * (HD + 1)    # V-plus-ones width: 12*65 = 780
HW6 = 6 * (HD + 1)     # 390: six heads of [V_h | 1]


def _build():
    nc = bacc.Bacc("TRN2", target_bir_lowering=False, debug=False, num_devices=B)

    xT16 = nc.dram_tensor("xT16", [D, NT], FP16, kind="ExternalInput")
    wqk = nc.dram_tensor("wqk", [D, 2 * D], FP16, kind="ExternalInput")
    # wv/bv are extended on the host with a zero-weight, bias-1.0 column per
    # head ([V_h | 1] layout) so the PV matmul also produces softmax
    # denominators; bqk[p, et] = bias of feature et*128+p
    wv = nc.dram_tensor("wv", [D, VP_W], FP16, kind="ExternalInput")
    bqk = nc.dram_tensor("bqk", [128, H], F32, kind="ExternalInput")
    bv = nc.dram_tensor("bv", [128, VP_W], FP16, kind="ExternalInput")
    out = nc.dram_tensor("out", [NT, D], F32, kind="ExternalOutput")

    with tile.TileContext(nc) as tc:
        with (
            tc.tile_pool(name="res", bufs=1) as res,          # persistent tensors
            tc.tile_pool(name="wstream", bufs=2) as wstream,  # streamed weights
            tc.tile_pool(name="work", bufs=3) as work,
            tc.tile_pool(name="expp", bufs=16) as expp,       # 2 chunks of exp tiles
            tc.tile_pool(name="mm", bufs=2, space="PSUM") as mmp,       # 4 banks
            tc.tile_pool(name="pvpool", bufs=2, space="PSUM") as pvpool,  # 2 banks
            tc.tile_pool(name="tpp", bufs=2, space="PSUM") as tpp,        # 2 banks
        ):
            xt16 = [res.tile([128, NT], FP16, tag=f"xt16_{k}", name=f"xt16_{k}") for k in range(KC)]
            qkt = [res.tile([128, NT], FP16, tag=f"qkt{e}", name=f"qkt{e}") for e in range(H)]
            vp = [res.tile([128, VP_W], FP16, tag=f"vp{t}", name=f"vp{t}") for t in range(8)]
            osb = [res.tile([128, D], F32, tag=f"osb{t}", name=f"osb{t}") for t in range(8)]
            bqk_sb = res.tile([128, H], F32, tag="bqk")
            bvv = res.tile([128, VP_W], FP16, tag="bvv")
            ident = res.tile([128, 128], FP16, tag="ident")

            make_identity(nc, ident[:])

            def dma_wqk(p):
                ts = [wstream.tile([128, 256], FP16, tag=f"wqk{k}",
                                   name=f"wqk{k}_{p}") for k in range(KC)]
                for k in range(KC):
                    nc.sync.dma_start(ts[k][:], wqk[k * 128:(k + 1) * 128,
                                                    p * 256:(p + 1) * 256])
                return ts

            def dma_wv(n):
                ts = [wstream.tile([128, HW6], FP16, tag=f"wv{k}",
                                   name=f"wv{k}_{n}") for k in range(KC)]
                for k in range(KC):
                    nc.sync.dma_start(ts[k][:], wv[k * 128:(k + 1) * 128,
                                                   n * HW6:(n + 1) * HW6])
                return ts

            def etile_proj(et, wt):
                # e-tile et: even = Q-pair, odd = K-pair of pair et//2; holds
                # head (et//2*2) features on partitions 0-63, head (..+1) on
                # 64-127, tokens along free dim
                i = et % 2
                ps = mmp.tile([128, NT], F32, tag="mm", name=f"psqk{et}")
                for tcn in range(2):
                    for k in range(KC):
                        nc.tensor.matmul(
                            ps[:, tcn * 512:(tcn + 1) * 512],
                            wt[k][:, i * 128:(i + 1) * 128],
                            xt16[k][:, tcn * 512:(tcn + 1) * 512],
                            start=(k == 0), stop=(k == KC - 1),
                            skip_group_check=True)
                nc.vector.tensor_scalar_add(qkt[et][:], ps[:], bqk_sb[:, et:et + 1])

            def vproj_unit(n, t, wvt):
                ps = pvpool.tile([128, 512], F32, tag="pvp", name=f"psv{n}_{t}")
                for k in range(KC):
                    nc.tensor.matmul(ps[:, 0:HW6],
                                     xt16[k][:, t * 128:(t + 1) * 128],
                                     wvt[k][:],
                                     start=(k == 0), stop=(k == KC - 1),
                                     skip_group_check=True)
                nc.vector.tensor_add(vp[t][:, n * HW6:(n + 1) * HW6],
                                     ps[:, 0:HW6], bvv[:, n * HW6:(n + 1) * HW6])

            def energy_kt(c, kt, exl):
                # energy^T[k, q] for both heads of the pair; exp via ScalarE
                # with fused *scale (no max-subtraction: |energy*scale| < ~2.5)
                p, qc = divmod(c, 2)
                eps = mmp.tile([128, NT], F32, tag="mm", name=f"eps{c}_{kt}")
                for i in range(2):
                    qrow = slice(i * HD, (i + 1) * HD)
                    nc.tensor.matmul(
                        eps[:, i * 512:(i + 1) * 512],
                        qkt[2 * p + 1][qrow, kt * 128:(kt + 1) * 128],
                        qkt[2 * p][qrow, qc * 512:(qc + 1) * 512],
                        start=True, stop=True, skip_group_check=True)
                et_sb = expp.tile([128, NT], FP16, tag="exp", name=f"ex{c}_{kt}")
                nc.scalar.activation(et_sb[:], eps[:],
                                     mybir.ActivationFunctionType.Exp,
                                     bias=0.0, scale=SCALE)
                exl.append(et_sb)

            def pv_kt(c, kt, exl, pvps):
                p, qc = divmod(c, 2)
                for i in range(2):
                    h = 2 * p + i
                    nc.tensor.matmul(
                        pvps[i][:],
                        vp[kt][:, h * (HD + 1):(h + 1) * (HD + 1)],
                        exl[kt][:, i * 512:(i + 1) * 512],
                        start=(kt == 0), stop=(kt == 7),
                        skip_group_check=True)

            def tail(c, pvps):
                # [d+1, q] -> fp16 -> PE-transpose -> normalize -> DMA out
                p, qc = divmod(c, 2)
                for i in range(2):
                    h = 2 * p + i
                    pvt = work.tile([HD + 1, 512], FP16, tag="pvt", name=f"pvt{c}_{i}")
                    nc.vector.tensor_copy(pvt[:], pvps[i][:])
                    tpt = tpp.tile([128, 512], FP16, tag="tp", name=f"tp{c}_{i}")
                    for st in range(4):
                        nc.tensor.transpose(tpt[:, st * 128:st * 128 + 65],
                                            pvt[:, st * 128:(st + 1) * 128],
                                            ident[0:HD + 1, 0:HD + 1])
                    rc = work.tile([128, 4], F32, tag="rc", name=f"rc{c}_{i}")
                    nc.vector.reciprocal(rc[:], tpt[:, HD:4 * 128:128])
                    for st in range(4):
                        tt = qc * 4 + st
                        nc.vector.tensor_scalar_mul(
                            osb[tt][:, h * HD:(h + 1) * HD],
                            tpt[:, st * 128:st * 128 + HD], rc[:, st:st + 1])
                for st in range(4):
                    tt = qc * 4 + st
                    nc.sync.dma_start(
                        out[tt * 128:(tt + 1) * 128, 2 * p * HD:(2 * p + 2) * HD],
                        osb[tt][:, 2 * p * HD:(2 * p + 2) * HD])

            # ---- preamble ----
            # DMA order: first pair's weights + x before everything else so
            # the first projection can start as early as possible
            wqk_t = {0: dma_wqk(0)}
            for k in range(KC):
                nc.sync.dma_start(xt16[k][:], xT16[k * 128:(k + 1) * 128, :])
            wv_t = [dma_wv(0), dma_wv(1)]
            nc.sync.dma_start(bqk_sb[:], bqk[:, :])
            nc.sync.dma_start(bvv[:], bv[:, :])
            wqk_t[1] = dma_wqk(1)

            # dummy matmuls on the resident identity tile during the initial
            # DMA wait: keeps the PE busy so the HAM clock gate reaches 2.4
            # GHz before real work arrives (~3.4us of sustained activity)
            for w in range(6):
                wps = tpp.tile([128, 512], F32, tag="tp", name=f"warm{w}")
                for r in range(4):
                    nc.tensor.matmul(wps[:, 0:128], ident[:, :], ident[:, :],
                                     start=True, stop=True,
                                     skip_group_check=True)

            etile_proj(0, wqk_t[0])
            etile_proj(1, wqk_t[0])

            # ---- main software pipeline over chunks ----
            # period c issues: energy+exp(c) [interleaved per kt with PV(c-1)],
            # e-tile projection c+2, and the normalize/output tail of c-1.
            # Period 0 uses the V projection (no exp dependency) as PE filler.
            ex = {}
            pvp_of = {}
            vproj_units = [(n, t) for n in range(2) for t in range(8)]
            for c in range(NCH + 1):
                p, qc = divmod(c, 2)
                if c < NCH:
                    # prefetch weights two e-tiles ahead, project one e-tile
                    et = c + 2
                    if et < H:
                        if (et % 2 == 0 and et // 2 + 1 < NPAIR
                                and (et // 2 + 1) not in wqk_t):
                            wqk_t[et // 2 + 1] = dma_wqk(et // 2 + 1)
                        etile_proj(et, wqk_t[et // 2])
                    ex[c] = []
                    if c >= 1:
                        pvp_of[c] = [
                            pvpool.tile([128, 512], F32, tag="pvp",
                                        name=f"pvp{c}_{i}")[0:HD + 1, :]
                            for i in range(2)]
                    for kt in range(8):
                        energy_kt(c, kt, ex[c])
                        if c == 0:
                            for n, t in vproj_units[2 * kt:2 * kt + 2]:
                                vproj_unit(n, t, wv_t[n])
                        else:
                            pv_kt(c - 1, kt, ex[c - 1], pvp_of[c])
                    if c >= 1:
                        tail(c - 1, pvp_of.pop(c))
                        del ex[c - 1]
                else:
                    # final period: PV + tail of the last chunk
                    pvps = [pvpool.tile([128, 512], F32, tag="pvp",
                                        name=f"pvp{c}_{i}")[0:HD + 1, :]
                            for i in range(2)]
                    for kt in range(8):
                        pv_kt(c - 1, kt, ex[c - 1], pvps)
                    tail(c - 1, pvps)

    nc.compile()
    return nc


_NC_CACHE = None


def _get_nc():
    global _NC_CACHE
    if _NC_CACHE is None:
        _NC_CACHE = _build()
    return _NC_CACHE


def _perm_indices():
    d3 = np.arange(HD) * 3
    qk_cols = []
    for p in range(NPAIR):
        for s in (0, 1):  # Q tile then K tile
            for h in (2 * p, 2 * p + 1):
                qk_cols.append(h * (HD * 3) + d3 + s)
    v_cols = [h * (HD * 3) + d3 + 2 for h in range(H)]
    return np.concatenate(qk_cols), np.concatenate(v_cols)


def make_in_maps(x, w_qkv, b_qkv):
    qk_idx, v_idx = _perm_indices()
    wqk = np.ascontiguousarray(w_qkv[:, qk_idx], dtype=np.float16)
    # [D, 780]: per head [V_h (64 cols) | zero col]; matching bias gets 1.0 in
    # the zero col so vp = x@wv + bv carries softmax-denominator ones
    wv = np.zeros((D, VP_W), dtype=np.float16)
    bv1 = np.zeros(VP_W, dtype=np.float32)
    wv_perm = np.asarray(w_qkv, dtype=np.float32)[:, v_idx]
    bv_perm = np.asarray(b_qkv, dtype=np.float32)[v_idx]
    for h in range(H):
        wv[:, h * (HD + 1):h * (HD + 1) + HD] = wv_perm[:, h * HD:(h + 1) * HD]
        bv1[h * (HD + 1):h * (HD + 1) + HD] = bv_perm[h * HD:(h + 1) * HD]
        bv1[h * (HD + 1) + HD] = 1.0
    # [128, H]: bias of QK e-tile et at partition p is bqk_perm[et*128 + p]
    bqk = np.ascontiguousarray(
        np.asarray(b_qkv, dtype=np.float32)[qk_idx].reshape(H, 128).T)
    bv = np.ascontiguousarray(np.broadcast_to(bv1, (128, VP_W)).astype(np.float16))
    return [
        {
            "xT16": np.ascontiguousarray(np.asarray(x[b], dtype=np.float16).T),
            "wqk": wqk, "wv": wv, "bqk": bqk, "bv": bv,
        }
        for b in range(B)
    ]


def kernel(x, w_qkv, b_qkv):
    nc = _get_nc()
    in_maps = make_in_maps(x, w_qkv, b_qkv)
    res = run_bass_kernel_spmd(nc, in_maps, core_ids=list(range(B)))
    return np.stack([res.results[b]["out"] for b in range(B)]).astype(np.float32)


# revision 12
# speedup vs baseline: 1.1506x; 1.1506x over previous
"""Trainium2 Bass kernel for batched multi-head self-attention.

Problem: x[8,1024,768], w_qkv[768,2304], b_qkv[2304] ->
         out[8,1024,768]  (12 heads, head_dim 64, scale 768**-0.5)

Sharding: data-parallel over batch; each of the 8 NeuronCores processes one
batch element end-to-end (no collectives).

Per-core pipeline, software-pipelined so the PE never waits on the Scalar
engine's exp (which otherwise rate-limits attention):
  1. Host pre-work: transpose x[b] -> xT16 [768,1024] fp16; permute w_qkv
     columns so QK features are grouped per head-pair and V features
     head-major with a ones column per head (softmax denominators fall out
     of the PV matmul).
  2. QK projection in [feature, token] orientation (fp16) -> Q^T/K^T tiles;
     V projection in [token, feature] orientation (fp16) -> [V|1] tiles.
  3. Attention runs as 12 chunks c=(pair, q-half).  Steady state issues, per
     chunk period: energy matmuls + exp for chunk c interleaved (per k-tile)
     with the PV matmuls of chunk c-1, so exp(c-1) results are ready exactly
     when PV(c-1) consumes them and the Tensor engine stays saturated (and
     the HAM clock gate stays at 2.4 GHz).  exp is written as fp16, making
     the PV moving operand full-rate.  The PV output [d+1, q] (denominator
     row included) is copied to fp16, PE-transposed back to [q, d] (fp16,
     1 cycle/row), normalized with one batched reciprocal per head, and the
     finished 128-token x 2-head block is DMAed out per chunk.
"""

import numpy as np

import concourse.mybir as mybir
import concourse.tile as tile
from concourse import bacc
from concourse.bass_utils import run_bass_kernel_spmd
from concourse.masks import make_identity

B, NT, D, H, HD = 8, 1024, 768, 12, 64
KC = D // 128          # 6 contraction chunks
NPAIR = H // 2         # 6 head pairs
NCH = 2 * NPAIR        # 12 chunks: (pair, q-half)
SCALE = float(D) ** -0.5
F32 = mybir.dt.float32
FP16 = mybir.dt.float16
VP_W = H * (HD + 1)    # V-plus-ones width: 12*65 = 780
HW6 = 6 * (HD + 1)     # 390: six heads of [V_h | 1]


def _build():
    nc = bacc.Bacc("TRN2", target_bir_lowering=False, debug=False, num_devices=B)

    xT16 = nc.dram_tensor("xT16", [D, NT], FP16, kind="ExternalInput")
    wqk = nc.dram_tensor("wqk", [D, 2 * D], FP16, kind="ExternalInput")
    # wv/bv are extended on the host with a zero-weight, bias-1.0 column per
    # head ([V_h | 1] layout) so the PV matmul also produces softmax
    # denominators; bqk[p, et] = bias of feature et*128+p
    wv = nc.dram_tensor("wv", [D, VP_W], FP16, kind="ExternalInput")
    bqk = nc.dram_tensor("bqk", [128, H], F32, kind="ExternalInput")
    bv = nc.dram_tensor("bv", [128, VP_W], FP16, kind="ExternalInput")
    out = nc.dram_tensor("out", [NT, D], F32, kind="ExternalOutput")

    with tile.TileContext(nc) as tc:
        with (
            tc.tile_pool(name="res", bufs=1) as res,          # persistent tensors
            tc.tile_pool(name="wstream", bufs=2) as wstream,  # streamed weights
            tc.tile_pool(name="work", bufs=3) as work,
            tc.tile_pool(name="expp", bufs=16) as expp,       # 2 chunks of exp tiles
            tc.tile_pool(name="mm", bufs=2, space="PSUM") as mmp,       # 4 banks
            tc.tile_pool(name="pvpool", bufs=2, space="PSUM") as pvpool,  # 2 banks
            tc.tile_pool(name="tpp", bufs=2, space="PSUM") as tpp,        # 2 banks
        ):
            xt16 = [res.tile([128, NT], FP16, tag=f"xt16_{k}", name=f"xt16_{k}") for k in range(KC)]
            qkt = [res.tile([128, NT], FP16, tag=f"qkt{e}", name=f"qkt{e}") for e in range(H)]
            vp = [res.tile([128, VP_W], FP16, tag=f"vp{t}", name=f"vp{t}") for t in range(8)]
            osb = [res.tile([128, D], F32, tag=f"osb{t}", name=f"osb{t}") for t in range(8)]
            bqk_sb = res.tile([128, H], F32, tag="bqk")
            bvv = res.tile([128, VP_W], FP16, tag="bvv")
            ident = res.tile([128, 128], FP16, tag="ident")

            make_identity(nc, ident[:])

            def dma_wqk(p):
                ts = [wstream.tile([128, 256], FP16, tag=f"wqk{k}",
                                   name=f"wqk{k}_{p}") for k in range(KC)]
                for k in range(KC):
                    nc.sync.dma_start(ts[k][:], wqk[k * 128:(k + 1) * 128,
                                                    p * 256:(p + 1) * 256])
                return ts

            def dma_wv(n):
                ts = [wstream.tile([128, HW6], FP16, tag=f"wv{k}",
                                   name=f"wv{k}_{n}") for k in range(KC)]
                for k in range(KC):
                    nc.sync.dma_start(ts[k][:], wv[k * 128:(k + 1) * 128,
                                                   n * HW6:(n + 1) * HW6])
                return ts

            def etile_proj(et, wt):
                # e-tile et: even = Q-pair, odd = K-pair of pair et//2; holds
                # head (et//2*2) features on partitions 0-63, head (..+1) on
                # 64-127, tokens along free dim
                i = et % 2
                ps = mmp.tile([128, NT], F32, tag="mm", name=f"psqk{et}")
                for tcn in range(2):
                    for k in range(KC):
                        nc.tensor.matmul(
                            ps[:, tcn * 512:(tcn + 1) * 512],
                            wt[k][:, i * 128:(i + 1) * 128],
                            xt16[k][:, tcn * 512:(tcn + 1) * 512],
                            start=(k == 0), stop=(k == KC - 1),
                            skip_group_check=True)
                nc.vector.tensor_scalar_add(qkt[et][:], ps[:], bqk_sb[:, et:et + 1])

            def vproj_unit(n, t, wvt):
                ps = pvpool.tile([128, 512], F32, tag="pvp", name=f"psv{n}_{t}")
                for k in range(KC):
                    nc.tensor.matmul(ps[:, 0:HW6],
                                     xt16[k][:, t * 128:(t + 1) * 128],
                                     wvt[k][:],
                                     start=(k == 0), stop=(k == KC - 1),
                                     skip_group_check=True)
                nc.vector.tensor_add(vp[t][:, n * HW6:(n + 1) * HW6],
                                     ps[:, 0:HW6], bvv[:, n * HW6:(n + 1) * HW6])

            def energy_kt(c, kt, exl):
                # energy^T[k, q] for both heads of the pair; exp via ScalarE
                # with fused *scale (no max-subtraction: |energy*scale| < ~2.5)
                p, qc = divmod(c, 2)
                eps = mmp.tile([128, NT], F32, tag="mm", name=f"eps{c}_{kt}")
                for i in range(2):
                    qrow = slice(i * HD, (i + 1) * HD)
                    nc.tensor.matmul(
                        eps[:, i * 512:(i + 1) * 512],
                        qkt[2 * p + 1][qrow, kt * 128:(kt + 1) * 128],
                        qkt[2 * p][qrow, qc * 512:(qc + 1) * 512],
                        start=True, stop=True, skip_group_check=True)
                et_sb = expp.tile([128, NT], FP16, tag="exp", name=f"ex{c}_{kt}")
                nc.scalar.activation(et_sb[:], eps[:],
                                     mybir.ActivationFunctionType.Exp,
                                     bias=0.0, scale=SCALE)
                exl.append(et_sb)

            def pv_kt(c, kt, exl, pvps):
                p, qc = divmod(c, 2)
                for i in range(2):
                    h = 2 * p + i
                    nc.tensor.matmul(
                        pvps[i][:],
                        vp[kt][:, h * (HD + 1):(h + 1) * (HD + 1)],
                        exl[kt][:, i * 512:(i + 1) * 512],
                        start=(kt == 0), stop=(kt == 7),
                        skip_group_check=True)

            def tail(c, pvps):
                # [d+1, q] -> fp16 -> PE-transpose -> normalize -> DMA out
                p, qc = divmod(c, 2)
                for i in range(2):
                    h = 2 * p + i
                    pvt = work.tile([HD + 1, 512], FP16, tag="pvt", name=f"pvt{c}_{i}")
                    nc.vector.tensor_copy(pvt[:], pvps[i][:])
                    tpt = tpp.tile([128, 512], FP16, tag="tp", name=f"tp{c}_{i}")
                    for st in range(4):
                        nc.tensor.transpose(tpt[:, st * 128:st * 128 + 65],
                                            pvt[:, st * 128:(st + 1) * 128],
                                            ident[0:HD + 1, 0:HD + 1])
                    rc = work.tile([128, 4], F32, tag="rc", name=f"rc{c}_{i}")
                    nc.vector.reciprocal(rc[:], tpt[:, HD:4 * 128:128])
                    for st in range(4):
                        tt = qc * 4 + st
                        nc.vector.tensor_scalar_mul(
                            osb[tt][:, h * HD:(h + 1) * HD],
                            tpt[:, st * 128:st * 128 + HD], rc[:, st:st + 1])
                for st in range(4):
                    tt = qc * 4 + st
                    nc.sync.dma_start(
                        out[tt * 128:(tt + 1) * 128, 2 * p * HD:(2 * p + 2) * HD],
                        osb[tt][:, 2 * p * HD:(2 * p + 2) * HD])

            # ---- preamble ----
            # DMA order: first pair's weights + x before everything else so
            # the first projection can start as early as possible
            wqk_t = {0: dma_wqk(0)}
            for k in range(KC):
                nc.sync.dma_start(xt16[k][:], xT16[k * 128:(k + 1) * 128, :])
            wv_t = [dma_wv(0), dma_wv(1)]
            nc.sync.dma_start(bqk_sb[:], bqk[:, :])
            nc.sync.dma_start(bvv[:], bv[:, :])
            wqk_t[1] = dma_wqk(1)

            # dummy matmuls on the resident identity tile during the initial
            # DMA wait: keeps the PE busy so the HAM clock gate reaches 2.4
            # GHz before real work arrives (~3.4us of sustained activity)
            for w in range(6):
                wps = tpp.tile([128, 512], F32, tag="tp", name=f"warm{w}")
                for r in range(4):
                    nc.tensor.matmul(wps[:, 0:128], ident[:, :], ident[:, :],
                                     start=True, stop=True,
                                     skip_group_check=True)

            etile_proj(0, wqk_t[0])
            etile_proj(1, wqk_t[0])

            # ---- main software pipeline over chunks ----
            # period c issues: energy+exp(c) [interleaved per kt with PV(c-1)],
            # e-tile projection c+2, and the normalize/output tail of c-1.
            # Period 0 uses the V projection (no exp dependency) as PE filler.
            ex = {}
            pvp_of = {}
            vproj_units = [(n, t) for n in range(2) for t in range(8)]
            for c in range(NCH + 1):
                p, qc = divmod(c, 2)
                if c < NCH:
                    ex[c] = []
                    if c >= 1:
                        pvp_of[c] = [
                            pvpool.tile([128, 512], F32, tag="pvp",
                                        name=f"pvp{c}_{i}")[0:HD + 1, :]
                            for i in range(2)]
                    for kt in range(8):
                        energy_kt(c, kt, ex[c])
                        if c == 0:
                            for n, t in vproj_units[2 * kt:2 * kt + 2]:
                                vproj_unit(n, t, wv_t[n])
                        else:
                            pv_kt(c - 1, kt, ex[c - 1], pvp_of[c])
                    if c >= 1:
                        tail(c - 1, pvp_of.pop(c))
                        del ex[c - 1]
                    # project the e-tile needed two periods ahead (weights
                    # were prefetched; issuing at period end keeps the PE off
                    # the critical path of this period's exp->PV chain)
                    et = c + 2
                    if et < H:
                        if (et % 2 == 0 and et // 2 + 1 < NPAIR
                                and (et // 2 + 1) not in wqk_t):
                            wqk_t[et // 2 + 1] = dma_wqk(et // 2 + 1)
                        etile_proj(et, wqk_t[et // 2])
                else:
                    # final period: PV + tail of the last chunk
                    pvps = [pvpool.tile([128, 512], F32, tag="pvp",
                                        name=f"pvp{c}_{i}")[0:HD + 1, :]
                            for i in range(2)]
                    for kt in range(8):
                        pv_kt(c - 1, kt, ex[c - 1], pvps)
                    tail(c - 1, pvps)

    nc.compile()
    return nc


_NC_CACHE = None


def _get_nc():
    global _NC_CACHE
    if _NC_CACHE is None:
        _NC_CACHE = _build()
    return _NC_CACHE


def _perm_indices():
    d3 = np.arange(HD) * 3
    qk_cols = []
    for p in range(NPAIR):
        for s in (0, 1):  # Q tile then K tile
            for h in (2 * p, 2 * p + 1):
                qk_cols.append(h * (HD * 3) + d3 + s)
    v_cols = [h * (HD * 3) + d3 + 2 for h in range(H)]
    return np.concatenate(qk_cols), np.concatenate(v_cols)


def make_in_maps(x, w_qkv, b_qkv):
    qk_idx, v_idx = _perm_indices()
    wqk = np.ascontiguousarray(w_qkv[:, qk_idx], dtype=np.float16)
    # [D, 780]: per head [V_h (64 cols) | zero col]; matching bias gets 1.0 in
    # the zero col so vp = x@wv + bv carries softmax-denominator ones
    wv = np.zeros((D, VP_W), dtype=np.float16)
    bv1 = np.zeros(VP_W, dtype=np.float32)
    wv_perm = np.asarray(w_qkv, dtype=np.float32)[:, v_idx]
    bv_perm = np.asarray(b_qkv, dtype=np.float32)[v_idx]
    for h in range(H):
        wv[:, h * (HD + 1):h * (HD + 1) + HD] = wv_perm[:, h * HD:(h + 1) * HD]
        bv1[h * (HD + 1):h * (HD + 1) + HD] = bv_perm[h * HD:(h + 1) * HD]
        bv1[h * (HD + 1) + HD] = 1.0
    # [128, H]: bias of QK e-tile et at partition p is bqk_perm[et*128 + p]
    bqk = np.ascontiguousarray(
        np.asarray(b_qkv, dtype=np.float32)[qk_idx].reshape(H, 128).T)
    bv = np.ascontiguousarray(np.broadcast_to(bv1, (128, VP_W)).astype(np.float16))
    return [
        {
            "xT16": np.ascontiguousarray(np.asarray(x[b], dtype=np.float16).T),
            "wqk": wqk, "wv": wv, "bqk": bqk, "bv": bv,
        }
        for b in range(B)
    ]


def kernel(x, w_qkv, b_qkv):
    nc = _get_nc()
    in_maps = make_in_maps(x, w_qkv, b_qkv)
    res = run_bass_kernel_spmd(nc, in_maps, core_ids=list(range(B)))
    return np.stack([res.results[b]["out"] for b in range(B)]).astype(np.float32)
